# revision 16
# baseline (speedup 1.0000x reference)
"""Additive-attention pooling kernel for 8 TRN2 NeuronCores.

reference:
    h     = tanh(x @ (W1+W2) + (b1+b2))      x: [B, T, D]
    score = h @ V + V_b                      [B, T, 1]
    attn  = softmax(score, axis=T)
    out   = sum_t attn * x                   [B, D]

Sharding: data-parallel over batch; each of the 8 cores gets B/8 = 8
examples (8 MB of fp32), weights replicated. No collectives.

Layout: token t = c*2048 + p*16 + i lands on partition p, tile i of
example c.  Token order within an example is permuted vs the reference
(softmax pooling is permutation-invariant), so each partition reads
contiguous 2-8 KB runs from HBM -- near line-rate DMA descriptors.

Per-core schedule:
  All x DMAs are issued upfront (x is SBUF-resident) as SWDGE casting
  transfers that land bf16 directly (f32 HBM -> bf16 SBUF), so there is
  no separate DVE cast stage.  Per tile: PE transposes x (xT), PE
  h-matmul with stationary Wc -> psum; ACT tanh(+bias) -> hT sbuf; PE
  score matmul (stationary hT, rhs v) -> psum column; ACT exp ->
  masked e-block column; PE context matmul REVERSED: stationary x_bf
  tile, rhs 8-col e-block -> accumulates ctx^T [d, 8] in one psum
  bank.  Denominators via a DVE reduce of the e-blocks + one 1-col
  matmul against ones.  Final: ctx^T -> sbuf -> PE transpose -> [8,128],
  reciprocal + row scale, one 4 KB DMA out.

V_b is omitted: softmax(score + c) == softmax(score) exactly.
Softmax runs without max-subtraction: |score| <= sum|V_u| ~ 9, exp is
safely within fp32 range.
"""

import os

import numpy as np

B, T, D, U = 64, 2048, 128, 128
N_CORES = 8
B_LOC = B // N_CORES          # 8 examples per core
N_TILE_EX = T // 128          # 16 token-tiles per example
N_TILES = B_LOC * N_TILE_EX   # 128 tiles per core

# SWDGE casting DMA (f32 HBM -> bf16 SBUF).  Fallback: HWDGE f32 DMA
# + DVE casts.
USE_DMA_CAST = bool(int(os.environ.get("KERNEL_DMA_CAST", "1")))

_nc = None
LAST_RESULT = None


def _build_nc():
    import concourse.bass as bass  # noqa: F401
    import concourse.mybir as mybir
    import concourse.tile as tile
    from concourse import bacc
    from concourse.masks import make_identity

    f32 = mybir.dt.float32
    bf16 = mybir.dt.bfloat16
    ACT = mybir.ActivationFunctionType
    ALU = mybir.AluOpType
    AX = mybir.AxisListType

    nc = bacc.Bacc("TRN2", target_bir_lowering=False, debug=False,
                   num_devices=N_CORES)

    x_d = nc.declare_dram_parameter("encoder_outputs", [B_LOC * T, D], f32,
                                    isOutput=False)
    w1_d = nc.declare_dram_parameter("W1_w", [D, U], f32, isOutput=False)
    b1_d = nc.declare_dram_parameter("W1_b", [U, 1], f32, isOutput=False)
    w2_d = nc.declare_dram_parameter("W2_w", [D, U], f32, isOutput=False)
    b2_d = nc.declare_dram_parameter("W2_b", [U, 1], f32, isOutput=False)
    v_d = nc.declare_dram_parameter("V_w", [U, 1], f32, isOutput=False)
    out_d = nc.declare_dram_parameter("out", [B_LOC, D], f32, isOutput=True)

    # token = c*T + p*16 + i  ->  [c][p][i][d]
    x_r = x_d.ap().rearrange("(c p i) d -> c p i d", c=B_LOC, p=128,
                             i=N_TILE_EX)

    # x DMA chunking per example: finer at the start (compute ramps
    # sooner) and at the end (shorter tail); whole-example in the middle
    # (8 KB descriptors, fewer SWDGE emissions).
    x_chunks = {0: 4, 1: 2, 2: 2, B_LOC - 1: 2}

    with tile.TileContext(nc) as tc:
        with (
            tc.tile_pool(name="consts", bufs=1) as consts,
            tc.tile_pool(name="big", bufs=1) as big,
            tc.tile_pool(name="ps_xt", bufs=2, space="PSUM") as ps_xt_pool,
            tc.tile_pool(name="ps_h", bufs=2, space="PSUM") as ps_h_pool,
            tc.tile_pool(name="ps_sc", bufs=1, space="PSUM") as ps_sc_pool,
            tc.tile_pool(name="ps_cx", bufs=1, space="PSUM") as ps_cx_pool,
        ):
            # ---- persistent buffers ----
            x_bf = big.tile([128, N_TILES * 128], bf16)     # 4 MB  [t, d]
            xT = big.tile([128, N_TILES * 128], bf16)       # 4 MB  [d, t]
            ht = big.tile([128, N_TILES * 128], bf16)       # 4 MB tanh(h)^T
            eb = big.tile([128, N_TILES * 8], bf16)         # masked e-blocks
            e_part = big.tile([128, 8], f32)
            e_tail = big.tile([128, 8], f32)
            e_all = big.tile([128, 8], f32)
            cxT_sb = big.tile([128, 8], f32)
            out_sb = big.tile([B_LOC, 128], f32)
            den_r = big.tile([B_LOC, 1], f32)

            x_bf_r = x_bf.rearrange("p (j d) -> p j d", d=128)
            eb_r = eb.rearrange("p (j e) -> p j e", e=8)
            eb_je = eb.rearrange("p (j e) -> p e j", e=8)

            ps_sc = ps_sc_pool.tile([128, 128], f32)   # score col per tile

            # ---- single identity first (transposes block on it), then
            # all x DMAs upfront; SWDGE casts f32 -> bf16 in flight.
            ident = consts.tile([128, 128], bf16)
            make_identity(nc, ident)

            xf_all = None
            if not USE_DMA_CAST:
                xf_all = big.tile([128, B_LOC * N_TILE_EX, 128], f32)

            for c in range(B_LOC):
                n_dma = x_chunks.get(c, 1)
                w_dma = N_TILE_EX // n_dma
                for h in range(n_dma):
                    lo, hi = w_dma * h, w_dma * (h + 1)
                    if USE_DMA_CAST:
                        nc.gpsimd.dma_start(
                            out=x_bf_r[:, 16 * c + lo:16 * c + hi],
                            in_=x_r[c][:, lo:hi])
                    else:
                        nc.sync.dma_start(
                            out=xf_all[:, 16 * c + lo:16 * c + hi],
                            in_=x_r[c][:, lo:hi])

            # f32 identity for the final [d,8]->[8,d] transpose; built on
            # the now-idle Q7 after all x descriptors are emitted.
            ident_f = consts.tile([128, 128], f32)
            make_identity(nc, ident_f)

            # ---- constants ride the otherwise-idle HWDGE path ----
            w1_sb = consts.tile([128, 128], f32)
            w2_sb = consts.tile([128, 128], f32)
            nc.sync.dma_start(out=w1_sb, in_=w1_d.ap())
            nc.sync.dma_start(out=w2_sb, in_=w2_d.ap())
            wc_bf = consts.tile([128, 128], bf16)   # [d, u] stationary
            nc.vector.tensor_add(wc_bf, w1_sb, w2_sb)

            b1_sb = consts.tile([128, 1], f32)
            b2_sb = consts.tile([128, 1], f32)
            nc.sync.dma_start(out=b1_sb, in_=b1_d.ap())
            nc.sync.dma_start(out=b2_sb, in_=b2_d.ap())
            b_sum = consts.tile([128, 1], f32)      # per-partition (=u) bias
            nc.vector.tensor_add(b_sum, b1_sb, b2_sb)

            vf_sb = consts.tile([128, 1], f32)
            nc.sync.dma_start(out=vf_sb, in_=v_d.ap())
            v_bf = consts.tile([128, 1], bf16)
            nc.vector.tensor_copy(v_bf, vf_sb)

            ones_col = consts.tile([128, 1], f32)
            nc.vector.memset(ones_col, 1.0)
            act_warm = consts.tile([128, 1], f32)

            # masked e-block columns start as zeros
            nc.vector.memset(eb, 0.0)

            # warm the ACT exp/tanh table set during the DMA ramp
            nc.scalar.activation(act_warm, ones_col, ACT.Exp)

            # one bank for ctx^T accum + final [8,128] + den
            ps_cx = ps_cx_pool.tile([128, 512], f32)

            for c in range(B_LOC):
                if not USE_DMA_CAST:
                    n_cast = 4 if c in (0, B_LOC - 1) else 2
                    w_cast = 16 // n_cast
                    for h in range(n_cast):
                        lo = 16 * c + w_cast * h
                        nc.vector.tensor_copy(
                            x_bf_r[:, lo:lo + w_cast],
                            xf_all[:, lo:lo + w_cast])

                if c >= B_LOC - 2:
                    # last examples: xbar DMA transposes (SBUF->SBUF,
                    # runs in the DMA idle window after the x stream;
                    # no PE work, no psum->sbuf copy)
                    for j in range(16 * c, 16 * (c + 1)):
                        nc.sync.dma_start_transpose(
                            out=xT[:, 128 * j:128 * (j + 1)],
                            in_=x_bf_r[:, j])
                else:
                    # transposes: 4 tiles per psum buffer
                    for q in range(4):
                        pxt = ps_xt_pool.tile([128, 512], bf16, tag="pxt")
                        for r in range(4):
                            j = 16 * c + 4 * q + r
                            nc.tensor.transpose(
                                pxt[:, 128 * r:128 * (r + 1)],
                                x_bf_r[:, j], ident)
                        s = 512 * (4 * c + q)
                        nc.vector.tensor_copy(xT[:, s:s + 512], pxt)

                # hT = tanh(Wc^T x^T + b) in [u, t] layout
                for g in range(2):
                    ph = ps_h_pool.tile([128, 1024], f32, tag="ph")
                    base = 2048 * c + 1024 * g
                    nc.tensor.matmul(ph[:, 0:512], lhsT=wc_bf,
                                     rhs=xT[:, base:base + 512])
                    if c == B_LOC - 1:
                        nc.scalar.activation(ht[:, base:base + 512],
                                             ph[:, 0:512], ACT.Tanh,
                                             bias=b_sum)
                        nc.tensor.matmul(ph[:, 512:1024], lhsT=wc_bf,
                                         rhs=xT[:, base + 512:base + 1024])
                        nc.scalar.activation(ht[:, base + 512:base + 1024],
                                             ph[:, 512:1024], ACT.Tanh,
                                             bias=b_sum)
                    else:
                        nc.tensor.matmul(ph[:, 512:1024], lhsT=wc_bf,
                                         rhs=xT[:, base + 512:base + 1024])
                        nc.scalar.activation(ht[:, base:base + 1024], ph,
                                             ACT.Tanh, bias=b_sum)

                # scores: one column of ps_sc per tile
                for i in range(N_TILE_EX):
                    j = 16 * c + i
                    nc.tensor.matmul(ps_sc[:, j:j + 1],
                                     lhsT=ht[:, 128 * j:128 * (j + 1)],
                                     rhs=v_bf)

                # e = exp(score) -> column c of each tile's e-block
                if c == B_LOC - 1:
                    nc.scalar.activation(eb_r[:, 16 * c:16 * c + 8, c],
                                         ps_sc[:, 16 * c:16 * c + 8],
                                         ACT.Exp)
                    nc.scalar.activation(eb_r[:, 16 * c + 8:16 * c + 16, c],
                                         ps_sc[:, 16 * c + 8:16 * c + 16],
                                         ACT.Exp)
                else:
                    nc.scalar.activation(eb_r[:, 16 * c:16 * c + 16, c],
                                         ps_sc[:, 16 * c:16 * c + 16],
                                         ACT.Exp)

                # ctx^T accumulation: stationary x tile, 8-col e stream
                for i in range(N_TILE_EX):
                    j = 16 * c + i
                    nc.tensor.matmul(ps_cx[:, 0:8], lhsT=x_bf_r[:, j],
                                     rhs=eb_r[:, j],
                                     start=(j == 0), stop=(j == N_TILES - 1))

                if c == B_LOC - 2:
                    # denominator partial over examples 0..6 (off the
                    # tail critical path)
                    nc.vector.tensor_reduce(e_part, eb_je[:, :, 0:112],
                                            AX.X, ALU.add)

            # ---- tail: denominator + final transpose/scale ----
            nc.vector.tensor_reduce(e_tail, eb_je[:, :, 112:128],
                                    AX.X, ALU.add)
            nc.vector.tensor_add(e_all, e_part, e_tail)
            nc.tensor.matmul(ps_cx[0:8, 448:449], lhsT=e_all, rhs=ones_col)

            nc.vector.tensor_copy(cxT_sb, ps_cx[:, 0:8])
            nc.tensor.transpose(ps_cx[0:8, 320:448], cxT_sb, ident_f)

            nc.vector.reciprocal(den_r, ps_cx[0:8, 448:449])
            nc.vector.tensor_scalar_mul(out_sb, ps_cx[0:8, 320:448], den_r)
            nc.sync.dma_start(out=out_d.ap(), in_=out_sb)

    nc.compile()
    return nc


def get_nc():
    global _nc
    if _nc is None:
        _nc = _build_nc()
    return _nc


def kernel(encoder_outputs, W1_w, W1_b, W2_w, W2_b, V_w, V_b):
    global LAST_RESULT
    from concourse.bass_utils import run_bass_kernel_spmd

    nc = get_nc()

    enc = np.ascontiguousarray(np.asarray(encoder_outputs, dtype=np.float32))
    rep = {
        "W1_w": np.ascontiguousarray(np.asarray(W1_w, np.float32)),
        "W1_b": np.ascontiguousarray(np.asarray(W1_b, np.float32).reshape(U, 1)),
        "W2_w": np.ascontiguousarray(np.asarray(W2_w, np.float32)),
        "W2_b": np.ascontiguousarray(np.asarray(W2_b, np.float32).reshape(U, 1)),
        "V_w": np.ascontiguousarray(np.asarray(V_w, np.float32).reshape(U, 1)),
    }
    in_maps = []
    for c in range(N_CORES):
        shard = enc[c * B_LOC:(c + 1) * B_LOC].reshape(B_LOC * T, D)
        in_maps.append({"encoder_outputs": np.ascontiguousarray(shard), **rep})

    trace = bool(int(os.environ.get("KERNEL_TRACE", "0")))
    LAST_RESULT = run_bass_kernel_spmd(
        nc, in_maps, core_ids=list(range(N_CORES)), trace=trace)
    out = np.concatenate(
        [LAST_RESULT.results[c]["out"] for c in range(N_CORES)], axis=0)
    return np.ascontiguousarray(out, dtype=np.float32)


# revision 18
# speedup vs baseline: 1.8197x; 1.8197x over previous
"""Additive-attention pooling kernel for 8 TRN2 NeuronCores.

reference:
    h     = tanh(x @ (W1+W2) + (b1+b2))      x: [B, T, D]
    score = h @ V + V_b                      [B, T, 1]
    attn  = softmax(score, axis=T)
    out   = sum_t attn * x                   [B, D]

Sharding: data-parallel over batch; each of the 8 cores gets B/8 = 8
examples (8 MB of fp32), weights replicated. No collectives.

Layout: token t = c*2048 + p*16 + i lands on partition p, tile i of
example c.  Token order within an example is permuted vs the reference
(softmax pooling is permutation-invariant), so each partition reads
contiguous 2-8 KB runs from HBM -- near line-rate DMA descriptors.

Per-core schedule:
  All x DMAs are issued upfront (x is SBUF-resident) as SWDGE casting
  transfers that land bf16 directly (f32 HBM -> bf16 SBUF), so there is
  no separate DVE cast stage.  Per tile: PE transposes x (xT), PE
  h-matmul with stationary Wc -> psum; ACT tanh(+bias) -> hT sbuf; PE
  score matmul (stationary hT, rhs v) -> psum column; ACT exp ->
  masked e-block column; PE context matmul REVERSED: stationary x_bf
  tile, rhs 8-col e-block -> accumulates ctx^T [d, 8] in one psum
  bank.  Denominators via a DVE reduce of the e-blocks + one 1-col
  matmul against ones.  Final: ctx^T -> sbuf -> PE transpose -> [8,128],
  reciprocal + row scale, one 4 KB DMA out.

V_b is omitted: softmax(score + c) == softmax(score) exactly.
Softmax runs without max-subtraction: |score| <= sum|V_u| ~ 9, exp is
safely within fp32 range.
"""

import os

import numpy as np

B, T, D, U = 64, 2048, 128, 128
N_CORES = 8
B_LOC = B // N_CORES          # 8 examples per core
N_TILE_EX = T // 128          # 16 token-tiles per example
N_TILES = B_LOC * N_TILE_EX   # 128 tiles per core

# SWDGE casting DMA (f32 HBM -> bf16 SBUF).  Fallback: HWDGE f32 DMA
# + DVE casts.
USE_DMA_CAST = bool(int(os.environ.get("KERNEL_DMA_CAST", "1")))

_nc = None
LAST_RESULT = None


def _build_nc():
    import concourse.bass as bass  # noqa: F401
    import concourse.mybir as mybir
    import concourse.tile as tile
    from concourse import bacc
    from concourse.masks import make_identity

    f32 = mybir.dt.float32
    bf16 = mybir.dt.bfloat16
    ACT = mybir.ActivationFunctionType
    ALU = mybir.AluOpType
    AX = mybir.AxisListType

    nc = bacc.Bacc("TRN2", target_bir_lowering=False, debug=False,
                   num_devices=N_CORES)

    x_d = nc.declare_dram_parameter("encoder_outputs", [B_LOC * T, D], f32,
                                    isOutput=False)
    w1_d = nc.declare_dram_parameter("W1_w", [D, U], f32, isOutput=False)
    b1_d = nc.declare_dram_parameter("W1_b", [U, 1], f32, isOutput=False)
    w2_d = nc.declare_dram_parameter("W2_w", [D, U], f32, isOutput=False)
    b2_d = nc.declare_dram_parameter("W2_b", [U, 1], f32, isOutput=False)
    v_d = nc.declare_dram_parameter("V_w", [U, 1], f32, isOutput=False)
    out_d = nc.declare_dram_parameter("out", [B_LOC, D], f32, isOutput=True)

    # token = c*T + p*16 + i  ->  [c][p][i][d]
    x_r = x_d.ap().rearrange("(c p i) d -> c p i d", c=B_LOC, p=128,
                             i=N_TILE_EX)

    # x DMA chunking per example: finer at the start (compute ramps
    # sooner) and at the end (shorter tail); whole-example in the middle
    # (8 KB descriptors, fewer SWDGE emissions).
    x_chunks = {0: 4, 1: 2, 2: 2, B_LOC - 1: 2}

    with tile.TileContext(nc) as tc:
        with (
            tc.tile_pool(name="consts", bufs=1) as consts,
            tc.tile_pool(name="big", bufs=1) as big,
            tc.tile_pool(name="ps_xt", bufs=2, space="PSUM") as ps_xt_pool,
            tc.tile_pool(name="ps_h", bufs=2, space="PSUM") as ps_h_pool,
            tc.tile_pool(name="ps_sc", bufs=1, space="PSUM") as ps_sc_pool,
            tc.tile_pool(name="ps_cx", bufs=1, space="PSUM") as ps_cx_pool,
        ):
            # ---- persistent buffers ----
            x_bf = big.tile([128, N_TILES * 128], bf16)     # 4 MB  [t, d]
            xT = big.tile([128, N_TILES * 128], bf16)       # 4 MB  [d, t]
            ht = big.tile([128, N_TILES * 128], bf16)       # 4 MB tanh(h)^T
            eb = big.tile([128, N_TILES * 8], bf16)         # masked e-blocks
            e_part = big.tile([128, 8], f32)
            e_tail = big.tile([128, 8], f32)
            e_all = big.tile([128, 8], f32)
            cxT_sb = big.tile([128, 8], f32)
            out_sb = big.tile([B_LOC, 128], f32)
            den_r = big.tile([B_LOC, 1], f32)

            x_bf_r = x_bf.rearrange("p (j d) -> p j d", d=128)
            eb_r = eb.rearrange("p (j e) -> p j e", e=8)
            eb_je = eb.rearrange("p (j e) -> p e j", e=8)

            ps_sc = ps_sc_pool.tile([128, 128], f32)   # score col per tile

            xf_all = None
            if not USE_DMA_CAST:
                xf_all = big.tile([128, B_LOC * N_TILE_EX, 128], f32)

            def issue_x(c, h, n_dma):
                w_dma = N_TILE_EX // n_dma
                lo, hi = w_dma * h, w_dma * (h + 1)
                if USE_DMA_CAST:
                    nc.gpsimd.dma_start(
                        out=x_bf_r[:, 16 * c + lo:16 * c + hi],
                        in_=x_r[c][:, lo:hi])
                else:
                    nc.sync.dma_start(
                        out=xf_all[:, 16 * c + lo:16 * c + hi],
                        in_=x_r[c][:, lo:hi])

            # first quarter of x before the identity build: the conveyor
            # starts immediately, the identity lands while q0 streams
            issue_x(0, 0, x_chunks[0])
            ident = consts.tile([128, 128], bf16)
            make_identity(nc, ident)
            for h in range(1, x_chunks[0]):
                issue_x(0, h, x_chunks[0])
            for c in range(1, B_LOC):
                n_dma = x_chunks.get(c, 1)
                for h in range(n_dma):
                    issue_x(c, h, n_dma)

            # f32 identity for the final [d,8]->[8,d] transpose; built on
            # the now-idle Q7 after all x descriptors are emitted.
            ident_f = consts.tile([128, 128], f32)
            make_identity(nc, ident_f)

            # ---- constants ride the otherwise-idle HWDGE path ----
            w1_sb = consts.tile([128, 128], f32)
            w2_sb = consts.tile([128, 128], f32)
            nc.sync.dma_start(out=w1_sb, in_=w1_d.ap())
            nc.sync.dma_start(out=w2_sb, in_=w2_d.ap())
            wc_bf = consts.tile([128, 128], bf16)   # [d, u] stationary
            nc.vector.tensor_add(wc_bf, w1_sb, w2_sb)

            b1_sb = consts.tile([128, 1], f32)
            b2_sb = consts.tile([128, 1], f32)
            nc.sync.dma_start(out=b1_sb, in_=b1_d.ap())
            nc.sync.dma_start(out=b2_sb, in_=b2_d.ap())
            b_sum = consts.tile([128, 1], f32)      # per-partition (=u) bias
            nc.vector.tensor_add(b_sum, b1_sb, b2_sb)

            vf_sb = consts.tile([128, 1], f32)
            nc.sync.dma_start(out=vf_sb, in_=v_d.ap())
            v_bf = consts.tile([128, 1], bf16)
            nc.vector.tensor_copy(v_bf, vf_sb)

            ones_col = consts.tile([128, 1], f32)
            nc.vector.memset(ones_col, 1.0)
            act_warm = consts.tile([128, 1], f32)

            # masked e-block columns start as zeros
            nc.vector.memset(eb, 0.0)

            # warm the ACT exp/tanh table set during the DMA ramp
            nc.scalar.activation(act_warm, ones_col, ACT.Exp)

            # one bank for ctx^T accum + final [8,128] + den
            ps_cx = ps_cx_pool.tile([128, 512], f32)

            for c in range(B_LOC):
                if not USE_DMA_CAST:
                    n_cast = 4 if c in (0, B_LOC - 1) else 2
                    w_cast = 16 // n_cast
                    for h in range(n_cast):
                        lo = 16 * c + w_cast * h
                        nc.vector.tensor_copy(
                            x_bf_r[:, lo:lo + w_cast],
                            xf_all[:, lo:lo + w_cast])

                # transposes: 4 tiles per psum buffer
                for q in range(4):
                    pxt = ps_xt_pool.tile([128, 512], bf16, tag="pxt")
                    for r in range(4):
                        j = 16 * c + 4 * q + r
                        nc.tensor.transpose(
                            pxt[:, 128 * r:128 * (r + 1)],
                            x_bf_r[:, j], ident)
                    s = 512 * (4 * c + q)
                    nc.vector.tensor_copy(xT[:, s:s + 512], pxt)

                # hT = tanh(Wc^T x^T + b) in [u, t] layout
                for g in range(2):
                    ph = ps_h_pool.tile([128, 1024], f32, tag="ph")
                    base = 2048 * c + 1024 * g
                    nc.tensor.matmul(ph[:, 0:512], lhsT=wc_bf,
                                     rhs=xT[:, base:base + 512])
                    if c == B_LOC - 1:
                        nc.scalar.activation(ht[:, base:base + 512],
                                             ph[:, 0:512], ACT.Tanh,
                                             bias=b_sum)
                        nc.tensor.matmul(ph[:, 512:1024], lhsT=wc_bf,
                                         rhs=xT[:, base + 512:base + 1024])
                        nc.scalar.activation(ht[:, base + 512:base + 1024],
                                             ph[:, 512:1024], ACT.Tanh,
                                             bias=b_sum)
                    else:
                        nc.tensor.matmul(ph[:, 512:1024], lhsT=wc_bf,
                                         rhs=xT[:, base + 512:base + 1024])
                        nc.scalar.activation(ht[:, base:base + 1024], ph,
                                             ACT.Tanh, bias=b_sum)

                # scores: one column of ps_sc per tile
                for i in range(N_TILE_EX):
                    j = 16 * c + i
                    nc.tensor.matmul(ps_sc[:, j:j + 1],
                                     lhsT=ht[:, 128 * j:128 * (j + 1)],
                                     rhs=v_bf)

                # e = exp(score) -> column c of each tile's e-block
                if c == B_LOC - 1:
                    nc.scalar.activation(eb_r[:, 16 * c:16 * c + 8, c],
                                         ps_sc[:, 16 * c:16 * c + 8],
                                         ACT.Exp)
                    nc.scalar.activation(eb_r[:, 16 * c + 8:16 * c + 16, c],
                                         ps_sc[:, 16 * c + 8:16 * c + 16],
                                         ACT.Exp)
                else:
                    nc.scalar.activation(eb_r[:, 16 * c:16 * c + 16, c],
                                         ps_sc[:, 16 * c:16 * c + 16],
                                         ACT.Exp)

                # ctx^T accumulation: stationary x tile, 8-col e stream
                for i in range(N_TILE_EX):
                    j = 16 * c + i
                    nc.tensor.matmul(ps_cx[:, 0:8], lhsT=x_bf_r[:, j],
                                     rhs=eb_r[:, j],
                                     start=(j == 0), stop=(j == N_TILES - 1))

                if c == B_LOC - 2:
                    # denominator partial over examples 0..6 (off the
                    # tail critical path)
                    nc.vector.tensor_reduce(e_part, eb_je[:, :, 0:112],
                                            AX.X, ALU.add)

            # ---- tail: denominator + final transpose/scale ----
            nc.vector.tensor_reduce(e_tail, eb_je[:, :, 112:128],
                                    AX.X, ALU.add)
            nc.vector.tensor_add(e_all, e_part, e_tail)
            nc.tensor.matmul(ps_cx[0:8, 448:449], lhsT=e_all, rhs=ones_col)

            nc.vector.tensor_copy(cxT_sb, ps_cx[:, 0:8])
            nc.tensor.transpose(ps_cx[0:8, 320:448], cxT_sb, ident_f)

            nc.vector.reciprocal(den_r, ps_cx[0:8, 448:449])
            nc.vector.tensor_scalar_mul(out_sb, ps_cx[0:8, 320:448], den_r)
            nc.sync.dma_start(out=out_d.ap(), in_=out_sb)

    nc.compile()
    return nc


def get_nc():
    global _nc
    if _nc is None:
        _nc = _build_nc()
    return _nc


def kernel(encoder_outputs, W1_w, W1_b, W2_w, W2_b, V_w, V_b):
    global LAST_RESULT
    from concourse.bass_utils import run_bass_kernel_spmd

    nc = get_nc()

    enc = np.ascontiguousarray(np.asarray(encoder_outputs, dtype=np.float32))
    rep = {
        "W1_w": np.ascontiguousarray(np.asarray(W1_w, np.float32)),
        "W1_b": np.ascontiguousarray(np.asarray(W1_b, np.float32).reshape(U, 1)),
        "W2_w": np.ascontiguousarray(np.asarray(W2_w, np.float32)),
        "W2_b": np.ascontiguousarray(np.asarray(W2_b, np.float32).reshape(U, 1)),
        "V_w": np.ascontiguousarray(np.asarray(V_w, np.float32).reshape(U, 1)),
    }
    in_maps = []
    for c in range(N_CORES):
        shard = enc[c * B_LOC:(c + 1) * B_LOC].reshape(B_LOC * T, D)
        in_maps.append({"encoder_outputs": np.ascontiguousarray(shard), **rep})

    trace = bool(int(os.environ.get("KERNEL_TRACE", "0")))
    LAST_RESULT = run_bass_kernel_spmd(
        nc, in_maps, core_ids=list(range(N_CORES)), trace=trace)
    out = np.concatenate(
        [LAST_RESULT.results[c]["out"] for c in range(N_CORES)], axis=0)
    return np.ascontiguousarray(out, dtype=np.float32)
